# revision 35
# baseline (speedup 1.0000x reference)
"""Trainium2 Bass kernel for nn_MultiHeadedAttentionWithGate.

Math (per molecule, validated against reference):
  The reference's reshapes are all flat views, so with u = "virtual row"
  (1024 per molecule), the computation is per-u over contiguous flat
  segments: K/V/M rows of 320 (10 nei x 32), X rows of 640 (10 x 64),
  q rows of 32.

Layout trick ("phase decomposition"): u = 4*g + r.  For fixed phase
r (0..3) and g on partitions, every tensor's u-row is a contiguous DRAM
segment, and the projections K/V/M[u-layout] decompose into matmuls over
X^T chunks whose row sets are stride-5 (rows 5g+d) -- an affine AP.
The 20 (d, fc) X^T chunks per 128-g tile are the transposes of the
[128, 2560] per-(mol, G) X tile chunked by 128 columns: chunk j has
(d, fc) = divmod(j, 4).  All softmax/max/mean reductions are then
per-partition (free-axis) ops.  The neighbor-mean enters only via a dot
with Wg[64:128]; that dot is folded into the PE pass as N=1 matmuls.

v3 structure:
  - X loaded with one big cast-DMA per (mol, G) (10 KB contiguous lines,
    SWDGE); first three loads + transposes pre-issued before the q path.
  - X^T produced by XBAR dma_start_transpose on the sync ring (the only
    traffic there besides consts, to keep HWDGE completion lanes free).
  - q path transposed on the PE (idle at startup, warms HAM).
  - Matmuls issued j-major (per stationary chunk).
  - Gate finalization (gd/inv/c2/out) deferred one molecule so the PE
    stream never waits on the DVE softmax chain.
  - softmax denominators via one batched DVE reduce per (mol, G).

Sharding: data-parallel over batch: 8 molecules per core x 8 cores.
"""

import sys

for _p in ("/opt/trn_rl_repo", "/root/.axon_site/_ro/trn_rl_repo"):
    if _p not in sys.path:
        sys.path.insert(0, _p)

from contextlib import ExitStack

import numpy as np

import concourse.bass as bass
import concourse.mybir as mybir
from concourse import bacc
from concourse.tile import TileContext

F16 = mybir.dt.float16
F32 = mybir.dt.float32
EXP = mybir.ActivationFunctionType.Exp
ADD = mybir.AluOpType.add
MAX = mybir.AluOpType.max
MULT = mybir.AluOpType.mult
AXL_X = mybir.AxisListType.X

N_CORES = 8
BM = 8          # molecules per core
A = 128         # atoms
NEI = 10
D = 256
D2 = 512


def build_nc(with_bias: bool, bg_val: float) -> bass.Bass:
    nc = bacc.Bacc("TRN2", target_bir_lowering=False)

    x_h = nc.declare_dram_parameter("x", [BM, A * NEI, D2], F32, isOutput=False)
    qin_h = nc.declare_dram_parameter("qin", [BM, A, D], F32, isOutput=False)
    # all f16 consts packed into one blob (one DMA, one HWDGE lane):
    # wcat [0:3072) | ident [3072:3200) | wq [3200:3712) | ssel [3712:3744)
    # | s2sel [3744:3872) | wgav [3872:3880)
    cb16_h = nc.declare_dram_parameter("cb16", [128, 3880], F16, isOutput=False)
    cb32_h = nc.declare_dram_parameter("cb32", [128, 64], F32, isOutput=False)
    if with_bias:
        bcat_h = nc.declare_dram_parameter("bcat", [1, 3, 256], F16, isOutput=False)
        bq_h = nc.declare_dram_parameter("bq", [1, 256], F16, isOutput=False)
        ones_h = nc.declare_dram_parameter("ones", [1, 128], F16, isOutput=False)
    out_h = nc.declare_dram_parameter("out", [BM, A, D], F32, isOutput=True)

    # flat per-molecule views: u = 4g + r (+ 512*G)
    # X: per (mol, G) partition-g line = 2560 contiguous f32 (4 phases x 640)
    xg = (x_h[:].rearrange("b n c -> b (n c)")
          .rearrange("b (G g t) -> b G g t", G=2, g=128, t=2560))
    # qin in u-layout, both G halves in one AP: [g, G, r, k]
    q6 = (qin_h[:].rearrange("b a c -> b (a c)")
          .rearrange("b (G g r k) -> b g G r k", G=2, g=128, r=4, k=32))
    o6 = (out_h[:].rearrange("b a c -> b (a c)")
          .rearrange("b (G g r k) -> b g G r k", G=2, g=128, r=4, k=32))

    with TileContext(nc) as tc, ExitStack() as ctx:
        consts = ctx.enter_context(tc.tile_pool(name="consts", bufs=1))
        sb_xg = ctx.enter_context(tc.tile_pool(name="xg", bufs=4))
        sb_xt = ctx.enter_context(tc.tile_pool(name="xt", bufs=4))
        sb_big = ctx.enter_context(tc.tile_pool(name="big", bufs=6))
        sb_ew = ctx.enter_context(tc.tile_pool(name="ew", bufs=6))
        sb_stash = ctx.enter_context(tc.tile_pool(name="stash", bufs=6))
        sb_q = ctx.enter_context(tc.tile_pool(name="qp", bufs=4))
        ps_km = ctx.enter_context(tc.tile_pool(name="pkm", bufs=2, space="PSUM"))
        ps_v = ctx.enter_context(tc.tile_pool(name="pv", bufs=3, space="PSUM"))
        ps_misc = ctx.enter_context(tc.tile_pool(name="pm", bufs=1, space="PSUM"))
        dram = ctx.enter_context(tc.tile_pool(name="dram", bufs=1, space="DRAM"))

        def cload(h, shape, dtype):
            t = consts.tile(shape, dtype, tag=h.name, name=h.name)
            nc.sync.dma_start(out=t, in_=h[:])
            return t

        cb16 = consts.tile([128, 3880], F16, tag="cb16", name="cb16")
        nc.sync.dma_start(out=cb16, in_=cb16_h[:])
        cb32 = consts.tile([128, 64], F32, tag="cb32", name="cb32")
        nc.sync.dma_start(out=cb32, in_=cb32_h[:])
        wcat_t = cb16[:, 0:3072].rearrange("p (a b) -> p a b", a=4)
        ident_t = cb16[:, 3072:3200]
        wq_t = cb16[:, 3200:3712].rearrange("p (a b) -> p a b", a=2)
        ssel_t = cb16[:, 3712:3744]
        s2sel_t = cb16[0:32, 3744:3872]
        wgav_t = cb16[:, 3872:3877]
        wgc_t = cb32[:, 0:32]
        wge_t = cb32[:, 32:64]
        if with_bias:
            bcat_t = cload(bcat_h, [1, 3, 256], F16)
            bq_t = cload(bq_h, [1, 256], F16)
            ones_t = cload(ones_h, [1, 128], F16)

        qdram = dram.tile([BM, A * D], F32)

        def load_x(mol, G, split=False):
            xbig = sb_xg.tile([128, 2560], F16, tag="xg", name="xbig")
            nc.gpsimd.dma_start(out=xbig, in_=xg[mol, G])
            xt = sb_xt.tile([128, 20, 128], F16, tag="xt", name="xt")
            if split:
                # per-phase transposes so the first matmuls start sooner
                for t in range(4):
                    nc.sync.dma_start_transpose(
                        out=xt[:, 5 * t:5 * t + 5, :],
                        in_=xbig[:, 640 * t:640 * (t + 1)])
            else:
                nc.sync.dma_start_transpose(out=xt, in_=xbig)
            return xt

        # prefetch X for the first units; the q path overlaps the loads
        xt_pre = {}
        for (mol, G) in ((0, 0), (0, 1), (1, 0), (1, 1)):
            xt_pre[(mol, G)] = load_x(mol, G, split=mol == 0)

        def emit_qpair(m):
            """q projections for mols m, m+1 (natural layout) -> DRAM."""
            qin2 = sb_q.tile([128, 2, 256], F16, tag="qin16", name="qin2")
            nc.gpsimd.dma_start(
                out=qin2, in_=qin_h[m:m + 2].rearrange("b a c -> a b c"))
            qtp = ps_misc.tile([128, 4, 128], F16, tag="pm", name="qtp")
            for w in range(4):
                nc.tensor.transpose(
                    qtp[:, w, :],
                    qin2[:, w // 2, 128 * (w % 2):128 * (w % 2 + 1)], ident_t)
            qT = sb_q.tile([128, 4, 128], F16, tag="qT", name="qT")
            nc.scalar.copy(out=qT, in_=qtp)
            qpsum = ps_misc.tile([128, 2, 256], F32, tag="pm", name="qpsum")
            for i in range(2):
                nc.tensor.matmul(qpsum[:, i, :], qT[:, 2 * i, :], wq_t[:, 0, :],
                                 start=True, stop=False)
                nc.tensor.matmul(qpsum[:, i, :], qT[:, 2 * i + 1, :],
                                 wq_t[:, 1, :],
                                 start=False, stop=not with_bias)
                if with_bias:
                    nc.tensor.matmul(qpsum[:, i, :], ones_t, bq_t,
                                     start=False, stop=True)
            qnat = sb_q.tile([128, 2, 256], F32, tag="qnat", name="qnat")
            nc.scalar.copy(out=qnat, in_=qpsum)
            nc.gpsimd.dma_start(
                out=qdram[m:m + 2].rearrange("b (a c) -> a b c", a=128),
                in_=qnat)

        emit_qpair(0)

        def emit_unit(mol, G, qu4):
            """kvm projections + softmax chain for one (mol, G) half.
            Returns (arawB, egB, egB16, raB) stash tuple."""
            xt = xt_pre.pop((mol, G))

            arawB = sb_stash.tile([128, 4, 32], F32, tag="arawB", name="arawB")
            emaxB = sb_ew.tile([128, 4, 32], F32, tag="emaxB", name="emaxB")
            adenB = sb_ew.tile([128, 4], F32, tag="adenB", name="adenB")
            ex4 = sb_ew.tile([128, 4, 10], F16, tag="ex4", name="ex4")
            gaveB = sb_ew.tile([128, 4], F32, tag="gaveB", name="gaveB")

            # gate term from the current atom: no kvm dependence, do it now
            curp = sb_ew.tile([128, 4, 32], F32, tag="curp", name="curp")
            nc.gpsimd.tensor_mul(
                curp, cur4[:, G],
                wgc_t.unsqueeze(1).broadcast_to([128, 4, 32]))
            gcurB = sb_ew.tile([128, 4], F32, tag="gcurB", name="gcurB")
            nc.vector.tensor_reduce(out=gcurB, in_=curp, axis=AXL_X, op=ADD)

            amul_pend = {}
            sv_pend = {}
            kvm = {}
            for j in range(20):
                d, fc = divmod(j, 4)
                lhsT = xt[:, j, :]
                # region A: r=d, W cols [64d, 256) -> out [0, 256-64d)
                # region B: r=d-1, W cols [0, 64d) -> out [320-64d, 320)
                regions = []
                if d <= 3:
                    regions.append((d, 0, 256 - 64 * d, 64 * d))
                if d >= 1:
                    regions.append((d - 1, 320 - 64 * d, 64 * d, 0))
                for (r, t0, wd, e0) in regions:
                    if r not in kvm:
                        kvm[r] = (
                            ps_km.tile([128, 2, 321], F32, tag="pkm",
                                       name=f"km{r}",
                                       padded_shape=[128, 2, 512]),
                            ps_v.tile([128, 320], F32, tag="pv",
                                      name=f"v{r}",
                                      padded_shape=[128, 512]))
                    km_t, v_t = kvm[r]
                    st = fc == 0 and t0 == 0
                    # region B (t0>0) at fc=3 is the tile's last write
                    sp = fc == 3 and t0 > 0 and not with_bias
                    for i, o in ((0, km_t[:, 0, t0:t0 + wd]),
                                 (1, v_t[:, t0:t0 + wd]),
                                 (2, km_t[:, 1, t0:t0 + wd])):
                        nc.tensor.matmul(
                            o, lhsT,
                            wcat_t[:, fc, 256 * i + e0:256 * i + e0 + wd],
                            start=st, stop=sp,
                            skip_group_check=True)
                    if with_bias and fc == 3:
                        for i, o in ((0, km_t[:, 0, t0:t0 + wd]),
                                     (1, v_t[:, t0:t0 + wd]),
                                     (2, km_t[:, 1, t0:t0 + wd])):
                            nc.tensor.matmul(
                                o, ones_t,
                                bcat_t[:, i, e0:e0 + wd],
                                start=False, stop=t0 > 0,
                                skip_group_check=True)
                # fold the neighbor-mean dot into the PE pass; its
                # accumulator lives in the km tile's bank-0 padding, which
                # the K group's start already bit-cleared (so never start)
                nc.tensor.matmul(
                    kvm[j // 5][0][:, 0, 320:321], lhsT,
                    wgav_t[:, j % 5:j % 5 + 1],
                    start=False, stop=False,
                    skip_group_check=True)

                # ---- per-phase elementwise, staggered software pipeline.
                # Stage layout at phase r: emit r's PSUM readers (smul,
                # v16, emax) so the kvm bank frees immediately, then phase
                # r-1's exp+amul (their inputs are a full phase old -> no
                # engine ever waits), then phase r-2's attention reduce.
                if j == 12:
                    u2 = 2 * mol + G + 4
                    if u2 < 2 * BM:
                        xt_pre[(u2 // 2, u2 % 2)] = load_x(u2 // 2, u2 % 2)
                if j in (7, 11, 15, 19):
                    r = (j - 7) // 4
                    km_t, v_t = kvm.pop(r)
                    smul = sb_big.tile([128, 320], F16, tag="smul",
                                       name="smul")
                    nc.vector.tensor_mul(
                        smul, km_t[:, 0, 0:320],
                        qu4[:, G, r, :].unsqueeze(1)
                        .broadcast_to([128, 10, 32]))
                    nc.vector.tensor_reduce(
                        out=emaxB[:, r, :],
                        in_=km_t[:, 1, 0:320]
                        .rearrange("p (j k) -> p k j", j=10),
                        axis=AXL_X, op=MAX)
                    nc.vector.tensor_copy(out=gaveB[:, r:r + 1],
                                          in_=km_t[:, 0, 320:321])
                    v16 = sb_big.tile([128, 320], F16, tag="v16", name="v16")
                    nc.scalar.copy(out=v16, in_=v_t[:, 0:320])
                    score = sb_ew.tile([128, 10], F32, tag="score",
                                       name="score")
                    nc.vector.tensor_reduce(
                        out=score,
                        in_=smul.rearrange("p (j k) -> p j k", j=10),
                        axis=AXL_X, op=ADD)
                    sv_pend[r] = (score, v16)
                    for rr in (r - 1,):
                        if rr in sv_pend:
                            sc_o, v16_o = sv_pend.pop(rr)
                            nc.scalar.activation(out=ex4[:, rr, :], in_=sc_o,
                                                 func=EXP)
                            amul = sb_big.tile([128, 320], F16, tag="amul",
                                               name="amul")
                            nc.gpsimd.tensor_mul(
                                amul, v16_o,
                                ex4[:, rr, :].unsqueeze(2)
                                .broadcast_to([128, 10, 32]))
                            amul_pend[rr] = amul
                    if r - 2 in amul_pend:
                        nc.vector.tensor_reduce(
                            out=arawB[:, r - 2, :],
                            in_=amul_pend.pop(r - 2)
                            .rearrange("p (j k) -> p k j", j=10),
                            axis=AXL_X, op=ADD)

            # ---- drain the staggered pipeline + gate logits ----
            sc_o, v16_o = sv_pend.pop(3)
            nc.scalar.activation(out=ex4[:, 3, :], in_=sc_o, func=EXP)
            amul = sb_big.tile([128, 320], F16, tag="amul", name="amul")
            nc.gpsimd.tensor_mul(
                amul, v16_o,
                ex4[:, 3, :].unsqueeze(2).broadcast_to([128, 10, 32]))
            amul_pend[3] = amul
            emaxp = sb_ew.tile([128, 4, 32], F32, tag="emaxp", name="emaxp")
            nc.gpsimd.tensor_mul(
                emaxp, emaxB,
                wge_t.unsqueeze(1).broadcast_to([128, 4, 32]))
            for rr in (2, 3):
                nc.vector.tensor_reduce(
                    out=arawB[:, rr, :],
                    in_=amul_pend.pop(rr).rearrange("p (j k) -> p k j", j=10),
                    axis=AXL_X, op=ADD)
            nc.vector.tensor_reduce(out=adenB, in_=ex4, axis=AXL_X, op=ADD)
            gemxB = sb_ew.tile([128, 4], F32, tag="gemxB", name="gemxB")
            nc.vector.tensor_reduce(out=gemxB, in_=emaxp, axis=AXL_X, op=ADD)
            gl1 = sb_ew.tile([128, 4], F32, tag="gl1", name="gl1")
            nc.vector.tensor_add(gl1, gcurB, gemxB)
            gl2 = sb_ew.tile([128, 4], F32, tag="gl2", name="gl2")
            nc.vector.tensor_add(gl2, gl1, gaveB)
            egB = sb_stash.tile([128, 4], F32, tag="egB", name="egB")
            nc.scalar.activation(out=egB, in_=gl2, func=EXP,
                                 bias=float(bg_val))
            egB16 = sb_stash.tile([128, 4], F16, tag="egB16", name="egB16")
            nc.scalar.copy(out=egB16, in_=egB)
            raB = sb_stash.tile([128, 4], F32, tag="raB", name="raB")
            nc.vector.reciprocal(out=raB, in_=adenB)
            return arawB, egB, egB16, raB

        def finalize(mol, st):
            """Cross-head gate softmax + output scaling + store for mol."""
            st0, st1 = st
            gd = ps_misc.tile([32, 4], F32, tag="pm", name="gd")
            for r in range(4):
                nc.tensor.matmul(gd[:, r:r + 1], ssel_t, st0[2][:, r:r + 1],
                                 start=True, stop=False)
                nc.tensor.matmul(gd[:, r:r + 1], ssel_t, st1[2][:, r:r + 1],
                                 start=False, stop=True)
            rg = sb_ew.tile([32, 4], F32, tag="rg", name="rg")
            nc.vector.reciprocal(out=rg, in_=gd)
            rg16 = sb_ew.tile([32, 4], F16, tag="rg16", name="rg16")
            nc.scalar.copy(out=rg16, in_=rg)
            inv = ps_misc.tile([128, 4], F32, tag="pm", name="inv")
            for r in range(4):
                nc.tensor.matmul(inv[:, r:r + 1], s2sel_t, rg16[:, r:r + 1],
                                 start=True, stop=True)
            outB = sb_stash.tile([128, 2, 4, 32], F32, tag="outB", name="outB")
            for gg, (ar_g, eg_g, eg16_g, ra_g) in ((0, st0), (1, st1)):
                t1 = sb_ew.tile([128, 4], F32, tag="t1", name="t1")
                nc.vector.tensor_mul(t1, inv, ra_g)
                c2 = sb_ew.tile([128, 4], F32, tag="c2", name="c2")
                nc.vector.tensor_mul(c2, t1, eg_g)
                nc.gpsimd.tensor_mul(
                    outB[:, gg], ar_g,
                    c2.unsqueeze(2).broadcast_to([128, 4, 32]))
            nc.scalar.dma_start(out=o6[mol], in_=outB)

        def load_q(mol):
            q_read = qdram[mol].rearrange(
                "(G g r k) -> g G r k", G=2, g=128, r=4, k=32)
            cur4 = sb_ew.tile([128, 2, 4, 32], F32, tag="cur4", name="cur4")
            nc.scalar.dma_start(out=cur4, in_=q6[mol])
            qu4 = sb_ew.tile([128, 2, 4, 32], F32, tag="qu4", name="qu4")
            nc.scalar.dma_start(out=qu4, in_=q_read)
            return cur4, qu4

        # remaining q pairs are emitted inside the early units so the PE
        # prologue stays short
        qpair_after = {(0, 0): 2, (0, 1): 4, (1, 0): 6}
        q_pre = {0: load_q(0)}
        pend = None
        for mol in range(BM):
            cur4, qu4 = q_pre.pop(mol)
            if mol + 1 < BM:
                q_pre[mol + 1] = load_q(mol + 1)
            st0 = emit_unit(mol, 0, qu4)
            if (mol, 0) in qpair_after:
                emit_qpair(qpair_after[(mol, 0)])
            st1 = emit_unit(mol, 1, qu4)
            if (mol, 1) in qpair_after:
                emit_qpair(qpair_after[(mol, 1)])
            if pend is not None:
                finalize(mol - 1, pend)
            pend = (st0, st1)
        finalize(BM - 1, pend)
    nc.finalize()
    return nc


def _prep_consts(Wq, bq, Wk, bk, Wv, bv, Wam, bam, Wg, bg):
    wcat = np.empty((128, 4, 768), np.float16)
    for i, W in enumerate((Wk, Wv, Wam)):
        for fc in range(4):
            wcat[:, fc, 256 * i:256 * (i + 1)] = W[128 * fc:128 * (fc + 1), :]
    wq = np.empty((128, 2, 256), np.float16)
    for fc in range(2):
        wq[:, fc, :] = Wq[128 * fc:128 * (fc + 1), :]
    p = np.arange(128)
    ssel = (p[:, None] % 32 == np.arange(32)[None, :]).astype(np.float16)
    s2sel = np.zeros((128, 128), np.float16)
    s2sel[0:32, :] = ssel.T
    wg = np.asarray(Wg[:, 0], np.float32)
    # wg_avc[floc, w] = Wg[64 + (floc % 64)] / NEI  (same for every w)
    wgav = np.empty((128, 8), np.float16)
    for w in range(8):
        wgav[:, w] = wg[64 + (np.arange(128) % 64)] / NEI
    cb16 = np.zeros((128, 3880), np.float16)
    cb16[:, 0:3072] = wcat.reshape(128, 3072)
    cb16[:, 3072:3200] = np.eye(128, dtype=np.float16)
    cb16[:, 3200:3712] = wq.reshape(128, 512)
    cb16[:, 3712:3744] = ssel
    cb16[:, 3744:3872] = s2sel
    cb16[:, 3872:3880] = wgav
    cb32 = np.empty((128, 64), np.float32)
    cb32[:, 0:32] = np.tile(wg[0:32], (128, 1))
    cb32[:, 32:64] = np.tile(wg[32:64], (128, 1))
    consts = {"cb16": cb16, "cb32": cb32}
    with_bias = any(np.any(np.asarray(b) != 0) for b in (bq, bk, bv, bam))
    if with_bias:
        bcat = np.stack([np.asarray(bk), np.asarray(bv), np.asarray(bam)]
                        ).astype(np.float16)[None, :, :].reshape(1, 3, 256)
        consts["bcat"] = bcat
        consts["bq"] = np.asarray(bq, np.float16).reshape(1, 256)
        consts["ones"] = np.ones((1, 128), np.float16)
    return consts, with_bias, float(np.asarray(bg).reshape(-1)[0])


_CACHE = {}
TRACE = False       # set by test.py for profiling runs
LAST_RESULTS = None  # BassKernelResults from the most recent run


def kernel(input_multihead, input_q, Wq, bq, Wk, bk, Wv, bv, Wam, bam, Wg, bg):
    from concourse.bass_utils import run_bass_kernel_spmd

    consts, with_bias, bg_val = _prep_consts(
        Wq, bq, Wk, bk, Wv, bv, Wam, bam, Wg, bg)

    key = (with_bias, bg_val)
    if key not in _CACHE:
        _CACHE[key] = build_nc(with_bias, bg_val)
    nc = _CACHE[key]

    x = np.ascontiguousarray(np.asarray(input_multihead, np.float32))
    q = np.ascontiguousarray(np.asarray(input_q, np.float32))
    in_maps = []
    for c in range(N_CORES):
        m = {"x": x[BM * c:BM * (c + 1)], "qin": q[BM * c:BM * (c + 1)]}
        m.update(consts)
        in_maps.append(m)

    res = run_bass_kernel_spmd(nc, in_maps, list(range(N_CORES)), trace=TRACE)
    global LAST_RESULTS
    LAST_RESULTS = res
    return np.concatenate([res.results[c]["out"] for c in range(N_CORES)],
                          axis=0)


# revision 36
# speedup vs baseline: 1.2148x; 1.2148x over previous
"""Trainium2 Bass kernel for nn_MultiHeadedAttentionWithGate.

Math (per molecule, validated against reference):
  The reference's reshapes are all flat views, so with u = "virtual row"
  (1024 per molecule), the computation is per-u over contiguous flat
  segments: K/V/M rows of 320 (10 nei x 32), X rows of 640 (10 x 64),
  q rows of 32.

Layout trick ("phase decomposition"): u = 4*g + r.  For fixed phase
r (0..3) and g on partitions, every tensor's u-row is a contiguous DRAM
segment (partition stride 2560 elems for X), and the projections
K/V/M[u-layout] decompose into matmuls over X^T chunks whose row sets
are stride-5 (rows 5g+d, d in 0..4) -- an affine AP.  The 20 (d, f-chunk)
X^T chunks per 128-g tile are the (f16) DMA-transposes of the 4 phases'
Xu tiles chunked by 128 columns.  All softmax/max/mean reductions are
then per-partition (free-axis) ops.  The neighbor-mean enters only via
a dot with Wg[64:128]; that dot is folded into the PE pass as 5 extra
N=1 matmuls per phase against the already-transposed X chunks.

Sharding: data-parallel over batch: 8 molecules per core x 8 cores.
"""

import sys

for _p in ("/opt/trn_rl_repo", "/root/.axon_site/_ro/trn_rl_repo"):
    if _p not in sys.path:
        sys.path.insert(0, _p)

from contextlib import ExitStack

import numpy as np

import concourse.bass as bass
import concourse.mybir as mybir
from concourse import bacc
from concourse.tile import TileContext

F16 = mybir.dt.float16
F32 = mybir.dt.float32
EXP = mybir.ActivationFunctionType.Exp
ADD = mybir.AluOpType.add
MAX = mybir.AluOpType.max
MULT = mybir.AluOpType.mult
AXL_X = mybir.AxisListType.X

N_CORES = 8
BM = 8          # molecules per core
A = 128         # atoms
NEI = 10
D = 256
D2 = 512


DEBUG = False


def build_nc(with_bias: bool, bg_val: float) -> bass.Bass:
    nc = bacc.Bacc("TRN2", target_bir_lowering=False)
    dbg = {}
    if DEBUG:
        for nm, shp in [("dbg_xt", [128, 128]), ("dbg_k", [128, 321]),
                        ("dbg_v", [128, 320]), ("dbg_m", [128, 320]),
                        ("dbg_score", [128, 10]), ("dbg_araw", [128, 4, 32]),
                        ("dbg_emax", [128, 4, 32]), ("dbg_gave", [128, 4]),
                        ("dbg_eg", [128, 4]), ("dbg_aden", [128, 4]),
                        ("dbg_qu", [128, 4, 32]), ("dbg_c2", [128, 4])]:
            dbg[nm] = nc.declare_dram_parameter(nm, shp, F32, isOutput=True)

    x_h = nc.declare_dram_parameter("x", [BM, A * NEI, D2], F32, isOutput=False)
    qin_h = nc.declare_dram_parameter("qin", [BM, A, D], F32, isOutput=False)
    wcat_h = nc.declare_dram_parameter("wcat", [128, 4, 768], F16, isOutput=False)
    ident_h = nc.declare_dram_parameter("ident", [128, 128], F16, isOutput=False)
    wq_h = nc.declare_dram_parameter("wq", [128, 2, 256], F16, isOutput=False)
    ssel_h = nc.declare_dram_parameter("ssel", [128, 32], F16, isOutput=False)
    s2sel_h = nc.declare_dram_parameter("s2sel", [32, 128], F16, isOutput=False)
    wgc_h = nc.declare_dram_parameter("wg_cur", [128, 32], F32, isOutput=False)
    wge_h = nc.declare_dram_parameter("wg_emax", [128, 32], F32, isOutput=False)
    wgav_h = nc.declare_dram_parameter("wg_avc", [128, 5], F16, isOutput=False)
    if with_bias:
        bcat_h = nc.declare_dram_parameter("bcat", [1, 3, 256], F16, isOutput=False)
        bq_h = nc.declare_dram_parameter("bq", [1, 256], F16, isOutput=False)
        ones_h = nc.declare_dram_parameter("ones", [1, 128], F16, isOutput=False)
    out_h = nc.declare_dram_parameter("out", [BM, A, D], F32, isOutput=True)

    # flat per-molecule views: u = 4g + r = 512*G + 4*p + r
    x5 = (x_h[:].rearrange("b n c -> b (n c)")
          .rearrange("b (g p t) -> b g p t", g=2, p=128, t=2560))
    q5 = (qin_h[:].rearrange("b a c -> b (a c)")
          .rearrange("b (g p r k) -> b g p r k", g=2, p=128, r=4, k=32))
    o5 = (out_h[:].rearrange("b a c -> b (a c)")
          .rearrange("b (g p r k) -> b g p r k", g=2, p=128, r=4, k=32))

    with TileContext(nc) as tc, ExitStack() as ctx:
        consts = ctx.enter_context(tc.tile_pool(name="consts", bufs=1))
        sb_x16 = ctx.enter_context(tc.tile_pool(name="x16", bufs=4))
        sb_xt = ctx.enter_context(tc.tile_pool(name="xt", bufs=44))
        sb_big = ctx.enter_context(tc.tile_pool(name="big", bufs=6))
        sb_ew = ctx.enter_context(tc.tile_pool(name="ew", bufs=6))
        sb_stash = ctx.enter_context(tc.tile_pool(name="stash", bufs=8))
        sb_q = ctx.enter_context(tc.tile_pool(name="qp", bufs=4))
        ps_proj = ctx.enter_context(tc.tile_pool(name="pp", bufs=2, space="PSUM"))
        ps_tp = ctx.enter_context(tc.tile_pool(name="pt", bufs=1, space="PSUM"))
        ps_misc = ctx.enter_context(tc.tile_pool(name="pm", bufs=1, space="PSUM"))
        dram = ctx.enter_context(tc.tile_pool(name="dram", bufs=1, space="DRAM"))

        def cload(h, shape, dtype):
            t = consts.tile(shape, dtype, tag=h.name)
            nc.sync.dma_start(out=t, in_=h[:])
            return t

        wcat_t = cload(wcat_h, [128, 4, 768], F16)
        ident_t = cload(ident_h, [128, 128], F16)
        wq_t = cload(wq_h, [128, 2, 256], F16)
        ssel_t = cload(ssel_h, [128, 32], F16)
        s2sel_t = cload(s2sel_h, [32, 128], F16)
        wgc_t = cload(wgc_h, [128, 32], F32)
        wge_t = cload(wge_h, [128, 32], F32)
        wgav_t = cload(wgav_h, [128, 5], F16)
        if with_bias:
            bcat_t = cload(bcat_h, [1, 3, 256], F16)
            bq_t = cload(bq_h, [1, 256], F16)
            ones_t = cload(ones_h, [1, 128], F16)

        qdram = dram.tile([BM, A * D], F32)

        # ---- all q projections up front (natural layout) -> DRAM ----
        for mol in range(BM):
            qin16 = sb_q.tile([128, 256], F16, tag="qin16")
            nc.gpsimd.dma_start(out=qin16, in_=qin_h[mol])
            qtp = ps_tp.tile([128, 2, 128], F16, tag="pt", name="qtp")
            for w in range(2):
                nc.tensor.transpose(qtp[:, w, :],
                                    qin16[:, 128 * w:128 * (w + 1)], ident_t)
            qT = sb_q.tile([128, 2, 128], F16, tag="qT")
            nc.scalar.copy(out=qT, in_=qtp)
            qpsum = ps_misc.tile([128, 256], F32, tag="pm")
            nc.tensor.matmul(qpsum, qT[:, 0, :], wq_t[:, 0, :],
                             start=True, stop=False)
            nc.tensor.matmul(qpsum, qT[:, 1, :], wq_t[:, 1, :],
                             start=False, stop=not with_bias)
            if with_bias:
                nc.tensor.matmul(qpsum, ones_t, bq_t, start=False, stop=True)
            qnat = sb_q.tile([128, 256], F32, tag="qnat")
            nc.scalar.copy(out=qnat, in_=qpsum)
            nc.scalar.dma_start(
                out=qdram[mol].rearrange("(a c) -> a c", a=128), in_=qnat)

        for mol in range(BM):
            q_read = qdram[mol].rearrange(
                "(g p r k) -> g p r k", g=2, p=128, r=4, k=32)

            for G in range(2):
                # ---- X load (cast f32->f16 in DMA) + PE transpose, with
                # 5 chunks batched per PSUM bank and one grouped copy ----
                xbig = sb_x16.tile([128, 2560], F16, tag="x16")
                nc.gpsimd.dma_start(out=xbig, in_=x5[mol, G])
                xu16 = [xbig[:, 640 * r:640 * (r + 1)] for r in range(4)]
                XT = {}
                for r in range(4):
                    tp = ps_tp.tile([128, 5, 128], F16, tag="pt", name="tp")
                    for w in range(5):
                        nc.tensor.transpose(
                            tp[:, w, :], xu16[r][:, 128 * w:128 * (w + 1)],
                            ident_t)
                    xtb = sb_xt.tile([128, 5, 128], F16, tag="xt")
                    nc.scalar.copy(out=xtb, in_=tp)
                    for w in range(5):
                        d, fc = divmod(5 * r + w, 4)
                        XT[(d, fc)] = xtb[:, w, :]

                cur4 = sb_ew.tile([128, 4, 32], F32, tag="cur4")
                nc.scalar.dma_start(out=cur4, in_=q5[mol, G])
                qu4 = sb_ew.tile([128, 4, 32], F32, tag="qu4")
                nc.scalar.dma_start(out=qu4, in_=q_read[G])

                arawB = sb_stash.tile([128, 4, 32], F32, tag="arawB")
                emaxB = sb_ew.tile([128, 4, 32], F32, tag="emaxB")
                gaveB = sb_ew.tile([128, 4], F32, tag="gaveB")
                adenB = sb_ew.tile([128, 4], F32, tag="adenB")
                pg = ps_misc.tile([128, 4], F32, tag="pm", name="pg")

                for r in range(4):
                    # ---- projections K|V|M into one 3-bank psum tile ----
                    wA = 256 - 64 * r
                    ranges = [(r, 0, wA, 64 * r), (r + 1, wA, 320 - wA, 0)]
                    kvm = ps_proj.tile([128, 3, 320], F32, tag="pp",
                                       padded_shape=[128, 3, 512])
                    for (d, t0, wd, e0) in ranges:
                        for fc in range(4):
                            st = fc == 0
                            sp = (fc == 3) and not with_bias
                            for i in range(3):
                                nc.tensor.matmul(
                                    kvm[:, i, t0:t0 + wd], XT[(d, fc)],
                                    wcat_t[:, fc, 256 * i + e0:256 * i + e0 + wd],
                                    start=st, stop=sp)
                            # fold the neighbor-mean dot into the PE pass
                            w_ave = 4 * d + fc - 5 * r
                            if 0 <= w_ave < 5:
                                nc.tensor.matmul(
                                    pg[:, r:r + 1], XT[(d, fc)],
                                    wgav_t[:, w_ave:w_ave + 1],
                                    start=(w_ave == 0), stop=(w_ave == 4),
                                    skip_group_check=True)
                        if with_bias:
                            for i in range(3):
                                nc.tensor.matmul(
                                    kvm[:, i, t0:t0 + wd], ones_t,
                                    bcat_t[:, i, e0:e0 + wd],
                                    start=False, stop=True)

                    # ---- per-phase elementwise ----
                    smul = sb_big.tile([128, 320], F16, tag="smul")
                    nc.vector.tensor_mul(
                        smul, kvm[:, 0, 0:320],
                        qu4[:, r, :].unsqueeze(1).broadcast_to([128, 10, 32]))
                    score = sb_ew.tile([128, 10], F32, tag="score")
                    nc.vector.tensor_reduce(
                        out=score, in_=smul.rearrange("p (j k) -> p j k", j=10),
                        axis=AXL_X, op=ADD)
                    ex = sb_ew.tile([128, 10], F16, tag="ex")
                    nc.scalar.activation(out=ex, in_=score, func=EXP,
                                         accum_out=adenB[:, r:r + 1])
                    v16 = sb_big.tile([128, 320], F16, tag="v16")
                    nc.scalar.copy(out=v16, in_=kvm[:, 1, 0:320])
                    amul = sb_big.tile([128, 320], F16, tag="amul")
                    nc.gpsimd.tensor_mul(
                        amul, v16,
                        ex.unsqueeze(2).broadcast_to([128, 10, 32]))
                    nc.vector.tensor_reduce(
                        out=arawB[:, r, :],
                        in_=amul.rearrange("p (j k) -> p k j", j=10),
                        axis=AXL_X, op=ADD)
                    nc.vector.tensor_reduce(
                        out=emaxB[:, r, :],
                        in_=kvm[:, 2, 0:320].rearrange("p (j k) -> p k j", j=10),
                        axis=AXL_X, op=MAX)
                    if DEBUG and mol == 0 and G == 0 and r == 0:
                        kc = sb_big.tile([128, 320], F32, tag="dbgk")
                        nc.vector.tensor_copy(out=kc, in_=kvm[:, 0, :])
                        nc.sync.dma_start(out=dbg["dbg_k"][:, :320], in_=kc)
                        vc = sb_big.tile([128, 320], F32, tag="dbgv")
                        nc.vector.tensor_copy(out=vc, in_=kvm[:, 1, :320])
                        nc.sync.dma_start(out=dbg["dbg_v"][:], in_=vc)
                        mc = sb_big.tile([128, 320], F32, tag="dbgm")
                        nc.vector.tensor_copy(out=mc, in_=kvm[:, 2, :320])
                        nc.sync.dma_start(out=dbg["dbg_m"][:], in_=mc)
                        nc.sync.dma_start(out=dbg["dbg_score"][:], in_=score)
                        xtc = sb_big.tile([128, 128], F32, tag="dbgxt")
                        nc.vector.tensor_copy(out=xtc, in_=XT[(0, 0)])
                        nc.sync.dma_start(out=dbg["dbg_xt"][:], in_=xtc)

                # ---- gate logits (batched over the 4 phases) ----
                nc.vector.tensor_copy(out=gaveB, in_=pg)
                curp = sb_ew.tile([128, 4, 32], F32, tag="curp")
                nc.gpsimd.tensor_mul(
                    curp, cur4,
                    wgc_t.unsqueeze(1).broadcast_to([128, 4, 32]))
                gcurB = sb_ew.tile([128, 4], F32, tag="gcurB")
                nc.vector.tensor_reduce(out=gcurB, in_=curp, axis=AXL_X, op=ADD)
                emaxp = sb_ew.tile([128, 4, 32], F32, tag="emaxp")
                nc.gpsimd.tensor_mul(
                    emaxp, emaxB,
                    wge_t.unsqueeze(1).broadcast_to([128, 4, 32]))
                gemxB = sb_ew.tile([128, 4], F32, tag="gemxB")
                nc.vector.tensor_reduce(out=gemxB, in_=emaxp, axis=AXL_X, op=ADD)
                gl1 = sb_ew.tile([128, 4], F32, tag="gl1")
                nc.vector.tensor_add(gl1, gcurB, gemxB)
                gl2 = sb_ew.tile([128, 4], F32, tag="gl2")
                nc.vector.tensor_add(gl2, gl1, gaveB)
                egB = sb_stash.tile([128, 4], F32, tag="egB")
                nc.scalar.activation(out=egB, in_=gl2, func=EXP,
                                     bias=float(bg_val))
                egB16 = sb_stash.tile([128, 4], F16, tag="egB16")
                nc.vector.tensor_copy(out=egB16, in_=egB)
                raB = sb_stash.tile([128, 4], F32, tag="raB")
                nc.vector.reciprocal(out=raB, in_=adenB)

                if DEBUG and mol == 0 and G == 0:
                    nc.sync.dma_start(out=dbg["dbg_araw"][:], in_=arawB)
                    nc.sync.dma_start(out=dbg["dbg_emax"][:], in_=emaxB)
                    nc.sync.dma_start(out=dbg["dbg_gave"][:], in_=gaveB)
                    egc = sb_ew.tile([128, 4], F32, tag="dbgeg")
                    nc.vector.tensor_copy(out=egc, in_=egB)
                    nc.sync.dma_start(out=dbg["dbg_eg"][:], in_=egc)
                    nc.sync.dma_start(out=dbg["dbg_aden"][:], in_=adenB)
                    nc.sync.dma_start(out=dbg["dbg_qu"][:], in_=qu4)

                if G == 0:
                    st0 = (arawB, egB, egB16, raB)
                else:
                    gd = ps_misc.tile([32, 4], F32, tag="pm", name="gd")
                    for r in range(4):
                        nc.tensor.matmul(gd[:, r:r + 1], ssel_t,
                                         st0[2][:, r:r + 1],
                                         start=True, stop=False)
                        nc.tensor.matmul(gd[:, r:r + 1], ssel_t,
                                         egB16[:, r:r + 1],
                                         start=False, stop=True)
                    rg = sb_ew.tile([32, 4], F32, tag="rg")
                    nc.vector.reciprocal(out=rg, in_=gd)
                    rg16 = sb_ew.tile([32, 4], F16, tag="rg16")
                    nc.vector.tensor_copy(out=rg16, in_=rg)
                    inv = ps_misc.tile([128, 4], F32, tag="pm", name="inv")
                    for r in range(4):
                        nc.tensor.matmul(inv[:, r:r + 1], s2sel_t,
                                         rg16[:, r:r + 1],
                                         start=True, stop=True)
                    c2B = {}
                    for gg, (ar_g, eg_g, eg16_g, ra_g) in (
                            (0, st0), (1, (arawB, egB, egB16, raB))):
                        t1 = sb_ew.tile([128, 4], F32, tag="t1", name="t1")
                        nc.vector.tensor_mul(t1, inv, ra_g)
                        c2B[gg] = sb_stash.tile([128, 4], F32, tag=f"c2B{gg}",
                                                name=f"c2B{gg}")
                        nc.vector.tensor_mul(c2B[gg], t1, eg_g)
                    if DEBUG and mol == 0:
                        nc.sync.dma_start(out=dbg["dbg_c2"][:], in_=c2B[0])
                    for gg, ar_g in ((0, st0[0]), (1, arawB)):
                        outB = sb_ew.tile([128, 4, 32], F32, tag="outB")
                        nc.gpsimd.tensor_mul(
                            outB, ar_g,
                            c2B[gg].unsqueeze(2).broadcast_to([128, 4, 32]))
                        nc.scalar.dma_start(out=o5[mol, gg], in_=outB)
    nc.finalize()
    return nc


def _prep_consts(Wq, bq, Wk, bk, Wv, bv, Wam, bam, Wg, bg):
    wcat = np.empty((128, 4, 768), np.float16)
    for i, W in enumerate((Wk, Wv, Wam)):
        for fc in range(4):
            wcat[:, fc, 256 * i:256 * (i + 1)] = W[128 * fc:128 * (fc + 1), :]
    wq = np.empty((128, 2, 256), np.float16)
    for fc in range(2):
        wq[:, fc, :] = Wq[128 * fc:128 * (fc + 1), :]
    p = np.arange(128)
    ssel = (p[:, None] % 32 == np.arange(32)[None, :]).astype(np.float16)
    s2sel = ssel.T.copy()
    wg = np.asarray(Wg[:, 0], np.float32)
    # wg_avc[floc, w] = Wg[64 + ((128*w + floc) % 64)] / NEI
    wgav = np.empty((128, 5), np.float32)
    for w in range(5):
        wgav[:, w] = wg[64 + (np.arange(128) % 64)] / NEI
    consts = {
        "wcat": wcat, "wq": wq,
        "ident": np.eye(128, dtype=np.float16),
        "ssel": ssel, "s2sel": s2sel,
        "wg_cur": np.tile(wg[0:32], (128, 1)).astype(np.float32),
        "wg_emax": np.tile(wg[32:64], (128, 1)).astype(np.float32),
        "wg_avc": wgav.astype(np.float16),
    }
    with_bias = any(np.any(np.asarray(b) != 0) for b in (bq, bk, bv, bam))
    if with_bias:
        bcat = np.stack([np.asarray(bk), np.asarray(bv), np.asarray(bam)]
                        ).astype(np.float16)[None, :, :].reshape(1, 3, 256)
        consts["bcat"] = bcat
        consts["bq"] = np.asarray(bq, np.float16).reshape(1, 256)
        consts["ones"] = np.ones((1, 128), np.float16)
    return consts, with_bias, float(np.asarray(bg).reshape(-1)[0])


_CACHE = {}
TRACE = False       # set by test.py for profiling runs
LAST_RESULTS = None  # BassKernelResults from the most recent run


def kernel(input_multihead, input_q, Wq, bq, Wk, bk, Wv, bv, Wam, bam, Wg, bg):
    from concourse.bass_utils import run_bass_kernel_spmd

    consts, with_bias, bg_val = _prep_consts(
        Wq, bq, Wk, bk, Wv, bv, Wam, bam, Wg, bg)

    key = (with_bias, bg_val)
    if key not in _CACHE:
        _CACHE[key] = build_nc(with_bias, bg_val)
    nc = _CACHE[key]

    x = np.ascontiguousarray(np.asarray(input_multihead, np.float32))
    q = np.ascontiguousarray(np.asarray(input_q, np.float32))
    in_maps = []
    for c in range(N_CORES):
        m = {"x": x[BM * c:BM * (c + 1)], "qin": q[BM * c:BM * (c + 1)]}
        m.update(consts)
        in_maps.append(m)

    res = run_bass_kernel_spmd(nc, in_maps, list(range(N_CORES)), trace=TRACE)
    global LAST_RESULTS
    LAST_RESULTS = res
    return np.concatenate([res.results[c]["out"] for c in range(N_CORES)],
                          axis=0)



# revision 38
# speedup vs baseline: 1.2681x; 1.0439x over previous
"""Trainium2 Bass kernel for nn_MultiHeadedAttentionWithGate.

Math (per molecule, validated against reference):
  The reference's reshapes are all flat views, so with u = "virtual row"
  (1024 per molecule), the computation is per-u over contiguous flat
  segments: K/V/M rows of 320 (10 nei x 32), X rows of 640 (10 x 64),
  q rows of 32.

Layout trick ("phase decomposition"): u = 4*g + r.  For fixed phase
r (0..3) and g on partitions, every tensor's u-row is a contiguous DRAM
segment (partition stride 2560 elems for X), and the projections
K/V/M[u-layout] decompose into matmuls over X^T chunks whose row sets
are stride-5 (rows 5g+d, d in 0..4) -- an affine AP.  The 20 (d, f-chunk)
X^T chunks per 128-g tile are the (f16) DMA-transposes of the 4 phases'
Xu tiles chunked by 128 columns.  All softmax/max/mean reductions are
then per-partition (free-axis) ops.  The neighbor-mean enters only via
a dot with Wg[64:128]; that dot is folded into the PE pass as 5 extra
N=1 matmuls per phase against the already-transposed X chunks.

Sharding: data-parallel over batch: 8 molecules per core x 8 cores.
"""

import sys

for _p in ("/opt/trn_rl_repo", "/root/.axon_site/_ro/trn_rl_repo"):
    if _p not in sys.path:
        sys.path.insert(0, _p)

from contextlib import ExitStack

import numpy as np

import concourse.bass as bass
import concourse.mybir as mybir
from concourse import bacc
from concourse.tile import TileContext

F16 = mybir.dt.float16
F32 = mybir.dt.float32
EXP = mybir.ActivationFunctionType.Exp
ADD = mybir.AluOpType.add
MAX = mybir.AluOpType.max
MULT = mybir.AluOpType.mult
AXL_X = mybir.AxisListType.X

N_CORES = 8
BM = 8          # molecules per core
A = 128         # atoms
NEI = 10
D = 256
D2 = 512


DEBUG = False


def build_nc(with_bias: bool, bg_val: float) -> bass.Bass:
    nc = bacc.Bacc("TRN2", target_bir_lowering=False)
    dbg = {}
    if DEBUG:
        for nm, shp in [("dbg_xt", [128, 128]), ("dbg_k", [128, 321]),
                        ("dbg_v", [128, 320]), ("dbg_m", [128, 320]),
                        ("dbg_score", [128, 10]), ("dbg_araw", [128, 4, 32]),
                        ("dbg_emax", [128, 4, 32]), ("dbg_gave", [128, 4]),
                        ("dbg_eg", [128, 4]), ("dbg_aden", [128, 4]),
                        ("dbg_qu", [128, 4, 32]), ("dbg_c2", [128, 4])]:
            dbg[nm] = nc.declare_dram_parameter(nm, shp, F32, isOutput=True)

    x_h = nc.declare_dram_parameter("x", [BM, A * NEI, D2], F32, isOutput=False)
    qin_h = nc.declare_dram_parameter("qin", [BM, A, D], F32, isOutput=False)
    wcat_h = nc.declare_dram_parameter("wcat", [128, 4, 768], F16, isOutput=False)
    ident_h = nc.declare_dram_parameter("ident", [128, 128], F16, isOutput=False)
    wq_h = nc.declare_dram_parameter("wq", [128, 2, 256], F16, isOutput=False)
    ssel_h = nc.declare_dram_parameter("ssel", [128, 32], F16, isOutput=False)
    s2sel_h = nc.declare_dram_parameter("s2sel", [32, 128], F16, isOutput=False)
    wgc_h = nc.declare_dram_parameter("wg_cur", [128, 32], F32, isOutput=False)
    wge_h = nc.declare_dram_parameter("wg_emax", [128, 32], F32, isOutput=False)
    wgav_h = nc.declare_dram_parameter("wg_avc", [128, 5], F16, isOutput=False)
    if with_bias:
        bcat_h = nc.declare_dram_parameter("bcat", [1, 3, 256], F16, isOutput=False)
        bq_h = nc.declare_dram_parameter("bq", [1, 256], F16, isOutput=False)
        ones_h = nc.declare_dram_parameter("ones", [1, 128], F16, isOutput=False)
    out_h = nc.declare_dram_parameter("out", [BM, A, D], F32, isOutput=True)

    # flat per-molecule views: u = 4g + r = 512*G + 4*p + r
    x5 = (x_h[:].rearrange("b n c -> b (n c)")
          .rearrange("b (g p t) -> b g p t", g=2, p=128, t=2560))
    q5 = (qin_h[:].rearrange("b a c -> b (a c)")
          .rearrange("b (g p r k) -> b g p r k", g=2, p=128, r=4, k=32))
    o5 = (out_h[:].rearrange("b a c -> b (a c)")
          .rearrange("b (g p r k) -> b g p r k", g=2, p=128, r=4, k=32))

    with TileContext(nc) as tc, ExitStack() as ctx:
        consts = ctx.enter_context(tc.tile_pool(name="consts", bufs=1))
        sb_x16 = ctx.enter_context(tc.tile_pool(name="x16", bufs=4))
        sb_xt = ctx.enter_context(tc.tile_pool(name="xt", bufs=44))
        sb_big = ctx.enter_context(tc.tile_pool(name="big", bufs=6))
        sb_ew = ctx.enter_context(tc.tile_pool(name="ew", bufs=6))
        sb_stash = ctx.enter_context(tc.tile_pool(name="stash", bufs=8))
        sb_q = ctx.enter_context(tc.tile_pool(name="qp", bufs=4))
        ps_proj = ctx.enter_context(tc.tile_pool(name="pp", bufs=2, space="PSUM"))
        ps_tp = ctx.enter_context(tc.tile_pool(name="pt", bufs=1, space="PSUM"))
        ps_misc = ctx.enter_context(tc.tile_pool(name="pm", bufs=1, space="PSUM"))
        dram = ctx.enter_context(tc.tile_pool(name="dram", bufs=1, space="DRAM"))

        def cload(h, shape, dtype):
            t = consts.tile(shape, dtype, tag=h.name)
            nc.sync.dma_start(out=t, in_=h[:])
            return t

        wcat_t = cload(wcat_h, [128, 4, 768], F16)
        ident_t = cload(ident_h, [128, 128], F16)
        wq_t = cload(wq_h, [128, 2, 256], F16)
        ssel_t = cload(ssel_h, [128, 32], F16)
        s2sel_t = cload(s2sel_h, [32, 128], F16)
        wgc_t = cload(wgc_h, [128, 32], F32)
        wge_t = cload(wge_h, [128, 32], F32)
        wgav_t = cload(wgav_h, [128, 5], F16)
        if with_bias:
            bcat_t = cload(bcat_h, [1, 3, 256], F16)
            bq_t = cload(bq_h, [1, 256], F16)
            ones_t = cload(ones_h, [1, 128], F16)

        qdram = dram.tile([BM, A * D], F32)
        pend = {}

        # ---- all q projections up front (natural layout) -> DRAM ----
        for mol in range(BM):
            qin16 = sb_q.tile([128, 256], F16, tag="qin16")
            nc.gpsimd.dma_start(out=qin16, in_=qin_h[mol])
            qtp = ps_tp.tile([128, 2, 128], F16, tag="pt", name="qtp")
            for w in range(2):
                nc.tensor.transpose(qtp[:, w, :],
                                    qin16[:, 128 * w:128 * (w + 1)], ident_t)
            qT = sb_q.tile([128, 2, 128], F16, tag="qT")
            nc.scalar.copy(out=qT, in_=qtp)
            qpsum = ps_misc.tile([128, 256], F32, tag="pm")
            nc.tensor.matmul(qpsum, qT[:, 0, :], wq_t[:, 0, :],
                             start=True, stop=False)
            nc.tensor.matmul(qpsum, qT[:, 1, :], wq_t[:, 1, :],
                             start=False, stop=not with_bias)
            if with_bias:
                nc.tensor.matmul(qpsum, ones_t, bq_t, start=False, stop=True)
            qnat = sb_q.tile([128, 256], F32, tag="qnat")
            nc.scalar.copy(out=qnat, in_=qpsum)
            nc.scalar.dma_start(
                out=qdram[mol].rearrange("(a c) -> a c", a=128), in_=qnat)

        for mol in range(BM):
            q_read = qdram[mol].rearrange(
                "(g p r k) -> g p r k", g=2, p=128, r=4, k=32)

            for G in range(2):
                # ---- X load (cast f32->f16 in DMA) + PE transpose, with
                # 5 chunks batched per PSUM bank and one grouped copy ----
                xbig = sb_x16.tile([128, 2560], F16, tag="x16")
                nc.gpsimd.dma_start(out=xbig, in_=x5[mol, G])
                xu16 = [xbig[:, 640 * r:640 * (r + 1)] for r in range(4)]
                XT = {}
                for r in range(4):
                    tp = ps_tp.tile([128, 5, 128], F16, tag="pt", name="tp")
                    for w in range(5):
                        nc.tensor.transpose(
                            tp[:, w, :], xu16[r][:, 128 * w:128 * (w + 1)],
                            ident_t)
                    xtb = sb_xt.tile([128, 5, 128], F16, tag="xt")
                    nc.scalar.copy(out=xtb, in_=tp)
                    for w in range(5):
                        d, fc = divmod(5 * r + w, 4)
                        XT[(d, fc)] = xtb[:, w, :]

                cur4 = sb_ew.tile([128, 4, 32], F32, tag="cur4")
                nc.scalar.dma_start(out=cur4, in_=q5[mol, G])
                qu4 = sb_ew.tile([128, 4, 32], F32, tag="qu4")
                nc.scalar.dma_start(out=qu4, in_=q_read[G])

                arawB = sb_stash.tile([128, 4, 32], F32, tag="arawB")
                emaxB = sb_ew.tile([128, 4, 32], F32, tag="emaxB")
                gaveB = sb_ew.tile([128, 4], F32, tag="gaveB")
                adenB = sb_ew.tile([128, 4], F32, tag="adenB")
                pg = ps_misc.tile([128, 4], F32, tag="pm", name="pg")

                for r in range(4):
                    # ---- projections K|V|M into one 3-bank psum tile ----
                    wA = 256 - 64 * r
                    ranges = [(r, 0, wA, 64 * r), (r + 1, wA, 320 - wA, 0)]
                    kvm = ps_proj.tile([128, 3, 320], F32, tag="pp",
                                       padded_shape=[128, 3, 512])
                    for (d, t0, wd, e0) in ranges:
                        for fc in range(4):
                            st = fc == 0
                            sp = (fc == 3) and not with_bias
                            for i in range(3):
                                nc.tensor.matmul(
                                    kvm[:, i, t0:t0 + wd], XT[(d, fc)],
                                    wcat_t[:, fc, 256 * i + e0:256 * i + e0 + wd],
                                    start=st, stop=sp)
                            # fold the neighbor-mean dot into the PE pass
                            w_ave = 4 * d + fc - 5 * r
                            if 0 <= w_ave < 5:
                                nc.tensor.matmul(
                                    pg[:, r:r + 1], XT[(d, fc)],
                                    wgav_t[:, w_ave:w_ave + 1],
                                    start=(w_ave == 0), stop=(w_ave == 4),
                                    skip_group_check=True)
                        if with_bias:
                            for i in range(3):
                                nc.tensor.matmul(
                                    kvm[:, i, t0:t0 + wd], ones_t,
                                    bcat_t[:, i, e0:e0 + wd],
                                    start=False, stop=True)

                    # ---- per-phase elementwise ----
                    smul = sb_big.tile([128, 320], F16, tag="smul")
                    nc.vector.tensor_mul(
                        smul, kvm[:, 0, 0:320],
                        qu4[:, r, :].unsqueeze(1).broadcast_to([128, 10, 32]))
                    score = sb_ew.tile([128, 10], F32, tag="score")
                    nc.vector.tensor_reduce(
                        out=score, in_=smul.rearrange("p (j k) -> p j k", j=10),
                        axis=AXL_X, op=ADD)
                    ex = sb_ew.tile([128, 10], F16, tag="ex")
                    nc.scalar.activation(out=ex, in_=score, func=EXP,
                                         accum_out=adenB[:, r:r + 1])
                    v16 = sb_big.tile([128, 320], F16, tag="v16")
                    nc.scalar.copy(out=v16, in_=kvm[:, 1, 0:320])
                    amul = sb_big.tile([128, 320], F16, tag="amul")
                    nc.gpsimd.tensor_mul(
                        amul, v16,
                        ex.unsqueeze(2).broadcast_to([128, 10, 32]))
                    nc.vector.tensor_reduce(
                        out=arawB[:, r, :],
                        in_=amul.rearrange("p (j k) -> p k j", j=10),
                        axis=AXL_X, op=ADD)
                    nc.vector.tensor_reduce(
                        out=emaxB[:, r, :],
                        in_=kvm[:, 2, 0:320].rearrange("p (j k) -> p k j", j=10),
                        axis=AXL_X, op=MAX)
                    if DEBUG and mol == 0 and G == 0 and r == 0:
                        kc = sb_big.tile([128, 320], F32, tag="dbgk")
                        nc.vector.tensor_copy(out=kc, in_=kvm[:, 0, :])
                        nc.sync.dma_start(out=dbg["dbg_k"][:, :320], in_=kc)
                        vc = sb_big.tile([128, 320], F32, tag="dbgv")
                        nc.vector.tensor_copy(out=vc, in_=kvm[:, 1, :320])
                        nc.sync.dma_start(out=dbg["dbg_v"][:], in_=vc)
                        mc = sb_big.tile([128, 320], F32, tag="dbgm")
                        nc.vector.tensor_copy(out=mc, in_=kvm[:, 2, :320])
                        nc.sync.dma_start(out=dbg["dbg_m"][:], in_=mc)
                        nc.sync.dma_start(out=dbg["dbg_score"][:], in_=score)
                        xtc = sb_big.tile([128, 128], F32, tag="dbgxt")
                        nc.vector.tensor_copy(out=xtc, in_=XT[(0, 0)])
                        nc.sync.dma_start(out=dbg["dbg_xt"][:], in_=xtc)

                # ---- gate logits (batched over the 4 phases) ----
                nc.vector.tensor_copy(out=gaveB, in_=pg)
                curp = sb_ew.tile([128, 4, 32], F32, tag="curp")
                nc.gpsimd.tensor_mul(
                    curp, cur4,
                    wgc_t.unsqueeze(1).broadcast_to([128, 4, 32]))
                gcurB = sb_ew.tile([128, 4], F32, tag="gcurB")
                nc.vector.tensor_reduce(out=gcurB, in_=curp, axis=AXL_X, op=ADD)
                emaxp = sb_ew.tile([128, 4, 32], F32, tag="emaxp")
                nc.gpsimd.tensor_mul(
                    emaxp, emaxB,
                    wge_t.unsqueeze(1).broadcast_to([128, 4, 32]))
                gemxB = sb_ew.tile([128, 4], F32, tag="gemxB")
                nc.vector.tensor_reduce(out=gemxB, in_=emaxp, axis=AXL_X, op=ADD)
                gl1 = sb_ew.tile([128, 4], F32, tag="gl1")
                nc.vector.tensor_add(gl1, gcurB, gemxB)
                gl2 = sb_ew.tile([128, 4], F32, tag="gl2")
                nc.vector.tensor_add(gl2, gl1, gaveB)
                egB = sb_stash.tile([128, 4], F32, tag="egB")
                nc.scalar.activation(out=egB, in_=gl2, func=EXP,
                                     bias=float(bg_val))
                egB16 = sb_stash.tile([128, 4], F16, tag="egB16")
                nc.vector.tensor_copy(out=egB16, in_=egB)
                raB = sb_stash.tile([128, 4], F32, tag="raB")
                nc.vector.reciprocal(out=raB, in_=adenB)

                if DEBUG and mol == 0 and G == 0:
                    nc.sync.dma_start(out=dbg["dbg_araw"][:], in_=arawB)
                    nc.sync.dma_start(out=dbg["dbg_emax"][:], in_=emaxB)
                    nc.sync.dma_start(out=dbg["dbg_gave"][:], in_=gaveB)
                    egc = sb_ew.tile([128, 4], F32, tag="dbgeg")
                    nc.vector.tensor_copy(out=egc, in_=egB)
                    nc.sync.dma_start(out=dbg["dbg_eg"][:], in_=egc)
                    nc.sync.dma_start(out=dbg["dbg_aden"][:], in_=adenB)
                    nc.sync.dma_start(out=dbg["dbg_qu"][:], in_=qu4)

                if G == 0:
                    st0 = (arawB, egB, egB16, raB)
                else:
                    pend[mol] = (st0, (arawB, egB, egB16, raB))

            # gate softmax + output for the previous molecule: deferred one
            # mol so the PE never waits on the current DVE softmax chain
            for fm in ([mol - 1] if mol >= 1 else []) + \
                    ([BM - 1] if mol == BM - 1 else []):
                if fm not in pend:
                    continue
                fst0, fst1 = pend.pop(fm)
                gd = ps_misc.tile([32, 4], F32, tag="pm", name="gd")
                for r in range(4):
                    nc.tensor.matmul(gd[:, r:r + 1], ssel_t,
                                     fst0[2][:, r:r + 1],
                                     start=True, stop=False)
                    nc.tensor.matmul(gd[:, r:r + 1], ssel_t,
                                     fst1[2][:, r:r + 1],
                                     start=False, stop=True)
                rg = sb_ew.tile([32, 4], F32, tag="rg")
                nc.vector.reciprocal(out=rg, in_=gd)
                rg16 = sb_ew.tile([32, 4], F16, tag="rg16")
                nc.vector.tensor_copy(out=rg16, in_=rg)
                inv = ps_misc.tile([128, 4], F32, tag="pm", name="inv")
                for r in range(4):
                    nc.tensor.matmul(inv[:, r:r + 1], s2sel_t,
                                     rg16[:, r:r + 1],
                                     start=True, stop=True)
                c2B = {}
                for gg, (ar_g, eg_g, eg16_g, ra_g) in (
                        (0, fst0), (1, fst1)):
                    t1 = sb_ew.tile([128, 4], F32, tag="t1", name="t1")
                    nc.vector.tensor_mul(t1, inv, ra_g)
                    c2B[gg] = sb_stash.tile([128, 4], F32, tag=f"c2B{gg}",
                                            name=f"c2B{gg}")
                    nc.vector.tensor_mul(c2B[gg], t1, eg_g)
                for gg, ar_g in ((0, fst0[0]), (1, fst1[0])):
                    outB = sb_ew.tile([128, 4, 32], F32, tag="outB")
                    nc.gpsimd.tensor_mul(
                        outB, ar_g,
                        c2B[gg].unsqueeze(2).broadcast_to([128, 4, 32]))
                    nc.scalar.dma_start(out=o5[fm, gg], in_=outB)
    nc.finalize()
    return nc


def _prep_consts(Wq, bq, Wk, bk, Wv, bv, Wam, bam, Wg, bg):
    wcat = np.empty((128, 4, 768), np.float16)
    for i, W in enumerate((Wk, Wv, Wam)):
        for fc in range(4):
            wcat[:, fc, 256 * i:256 * (i + 1)] = W[128 * fc:128 * (fc + 1), :]
    wq = np.empty((128, 2, 256), np.float16)
    for fc in range(2):
        wq[:, fc, :] = Wq[128 * fc:128 * (fc + 1), :]
    p = np.arange(128)
    ssel = (p[:, None] % 32 == np.arange(32)[None, :]).astype(np.float16)
    s2sel = ssel.T.copy()
    wg = np.asarray(Wg[:, 0], np.float32)
    # wg_avc[floc, w] = Wg[64 + ((128*w + floc) % 64)] / NEI
    wgav = np.empty((128, 5), np.float32)
    for w in range(5):
        wgav[:, w] = wg[64 + (np.arange(128) % 64)] / NEI
    consts = {
        "wcat": wcat, "wq": wq,
        "ident": np.eye(128, dtype=np.float16),
        "ssel": ssel, "s2sel": s2sel,
        "wg_cur": np.tile(wg[0:32], (128, 1)).astype(np.float32),
        "wg_emax": np.tile(wg[32:64], (128, 1)).astype(np.float32),
        "wg_avc": wgav.astype(np.float16),
    }
    with_bias = any(np.any(np.asarray(b) != 0) for b in (bq, bk, bv, bam))
    if with_bias:
        bcat = np.stack([np.asarray(bk), np.asarray(bv), np.asarray(bam)]
                        ).astype(np.float16)[None, :, :].reshape(1, 3, 256)
        consts["bcat"] = bcat
        consts["bq"] = np.asarray(bq, np.float16).reshape(1, 256)
        consts["ones"] = np.ones((1, 128), np.float16)
    return consts, with_bias, float(np.asarray(bg).reshape(-1)[0])


_CACHE = {}
TRACE = False       # set by test.py for profiling runs
LAST_RESULTS = None  # BassKernelResults from the most recent run


def kernel(input_multihead, input_q, Wq, bq, Wk, bk, Wv, bv, Wam, bam, Wg, bg):
    from concourse.bass_utils import run_bass_kernel_spmd

    consts, with_bias, bg_val = _prep_consts(
        Wq, bq, Wk, bk, Wv, bv, Wam, bam, Wg, bg)

    key = (with_bias, bg_val)
    if key not in _CACHE:
        _CACHE[key] = build_nc(with_bias, bg_val)
    nc = _CACHE[key]

    x = np.ascontiguousarray(np.asarray(input_multihead, np.float32))
    q = np.ascontiguousarray(np.asarray(input_q, np.float32))
    in_maps = []
    for c in range(N_CORES):
        m = {"x": x[BM * c:BM * (c + 1)], "qin": q[BM * c:BM * (c + 1)]}
        m.update(consts)
        in_maps.append(m)

    res = run_bass_kernel_spmd(nc, in_maps, list(range(N_CORES)), trace=TRACE)
    global LAST_RESULTS
    LAST_RESULTS = res
    return np.concatenate([res.results[c]["out"] for c in range(N_CORES)],
                          axis=0)



# revision 39
# speedup vs baseline: 1.2736x; 1.0044x over previous
"""Trainium2 Bass kernel for nn_MultiHeadedAttentionWithGate.

Math (per molecule, validated against reference):
  The reference's reshapes are all flat views, so with u = "virtual row"
  (1024 per molecule), the computation is per-u over contiguous flat
  segments: K/V/M rows of 320 (10 nei x 32), X rows of 640 (10 x 64),
  q rows of 32.

Layout trick ("phase decomposition"): u = 4*g + r.  For fixed phase
r (0..3) and g on partitions, every tensor's u-row is a contiguous DRAM
segment (partition stride 2560 elems for X), and the projections
K/V/M[u-layout] decompose into matmuls over X^T chunks whose row sets
are stride-5 (rows 5g+d, d in 0..4) -- an affine AP.  The 20 (d, f-chunk)
X^T chunks per 128-g tile are the (f16) DMA-transposes of the 4 phases'
Xu tiles chunked by 128 columns.  All softmax/max/mean reductions are
then per-partition (free-axis) ops.  The neighbor-mean enters only via
a dot with Wg[64:128]; that dot is folded into the PE pass as 5 extra
N=1 matmuls per phase against the already-transposed X chunks.

Sharding: data-parallel over batch: 8 molecules per core x 8 cores.
"""

import sys

for _p in ("/opt/trn_rl_repo", "/root/.axon_site/_ro/trn_rl_repo"):
    if _p not in sys.path:
        sys.path.insert(0, _p)

from contextlib import ExitStack

import numpy as np

import concourse.bass as bass
import concourse.mybir as mybir
from concourse import bacc
from concourse.tile import TileContext

F16 = mybir.dt.float16
F32 = mybir.dt.float32
EXP = mybir.ActivationFunctionType.Exp
ADD = mybir.AluOpType.add
MAX = mybir.AluOpType.max
MULT = mybir.AluOpType.mult
AXL_X = mybir.AxisListType.X

N_CORES = 8
BM = 8          # molecules per core
A = 128         # atoms
NEI = 10
D = 256
D2 = 512


DEBUG = False


def build_nc(with_bias: bool, bg_val: float) -> bass.Bass:
    nc = bacc.Bacc("TRN2", target_bir_lowering=False)
    dbg = {}
    if DEBUG:
        for nm, shp in [("dbg_xt", [128, 128]), ("dbg_k", [128, 321]),
                        ("dbg_v", [128, 320]), ("dbg_m", [128, 320]),
                        ("dbg_score", [128, 10]), ("dbg_araw", [128, 4, 32]),
                        ("dbg_emax", [128, 4, 32]), ("dbg_gave", [128, 4]),
                        ("dbg_eg", [128, 4]), ("dbg_aden", [128, 4]),
                        ("dbg_qu", [128, 4, 32]), ("dbg_c2", [128, 4])]:
            dbg[nm] = nc.declare_dram_parameter(nm, shp, F32, isOutput=True)

    x_h = nc.declare_dram_parameter("x", [BM, A * NEI, D2], F32, isOutput=False)
    qin_h = nc.declare_dram_parameter("qin", [BM, A, D], F32, isOutput=False)
    wcat_h = nc.declare_dram_parameter("wcat", [128, 4, 768], F16, isOutput=False)
    ident_h = nc.declare_dram_parameter("ident", [128, 128], F16, isOutput=False)
    wq_h = nc.declare_dram_parameter("wq", [128, 2, 256], F16, isOutput=False)
    ssel_h = nc.declare_dram_parameter("ssel", [128, 32], F16, isOutput=False)
    s2sel_h = nc.declare_dram_parameter("s2sel", [32, 128], F16, isOutput=False)
    wgc_h = nc.declare_dram_parameter("wg_cur", [128, 32], F32, isOutput=False)
    wge_h = nc.declare_dram_parameter("wg_emax", [128, 32], F32, isOutput=False)
    wgav_h = nc.declare_dram_parameter("wg_avc", [128, 5], F16, isOutput=False)
    if with_bias:
        bcat_h = nc.declare_dram_parameter("bcat", [1, 3, 256], F16, isOutput=False)
        bq_h = nc.declare_dram_parameter("bq", [1, 256], F16, isOutput=False)
        ones_h = nc.declare_dram_parameter("ones", [1, 128], F16, isOutput=False)
    out_h = nc.declare_dram_parameter("out", [BM, A, D], F32, isOutput=True)

    # flat per-molecule views: u = 4g + r = 512*G + 4*p + r
    x5 = (x_h[:].rearrange("b n c -> b (n c)")
          .rearrange("b (g p t) -> b g p t", g=2, p=128, t=2560))
    q5 = (qin_h[:].rearrange("b a c -> b (a c)")
          .rearrange("b (g p r k) -> b g p r k", g=2, p=128, r=4, k=32))
    o5 = (out_h[:].rearrange("b a c -> b (a c)")
          .rearrange("b (g p r k) -> b g p r k", g=2, p=128, r=4, k=32))

    with TileContext(nc) as tc, ExitStack() as ctx:
        consts = ctx.enter_context(tc.tile_pool(name="consts", bufs=1))
        sb_x16 = ctx.enter_context(tc.tile_pool(name="x16", bufs=4))
        sb_xt = ctx.enter_context(tc.tile_pool(name="xt", bufs=44))
        sb_big = ctx.enter_context(tc.tile_pool(name="big", bufs=6))
        sb_ew = ctx.enter_context(tc.tile_pool(name="ew", bufs=6))
        sb_stash = ctx.enter_context(tc.tile_pool(name="stash", bufs=8))
        sb_q = ctx.enter_context(tc.tile_pool(name="qp", bufs=4))
        ps_proj = ctx.enter_context(tc.tile_pool(name="pp", bufs=2, space="PSUM"))
        ps_tp = ctx.enter_context(tc.tile_pool(name="pt", bufs=1, space="PSUM"))
        ps_misc = ctx.enter_context(tc.tile_pool(name="pm", bufs=1, space="PSUM"))
        dram = ctx.enter_context(tc.tile_pool(name="dram", bufs=1, space="DRAM"))

        def cload(h, shape, dtype):
            t = consts.tile(shape, dtype, tag=h.name)
            nc.sync.dma_start(out=t, in_=h[:])
            return t

        wcat_t = cload(wcat_h, [128, 4, 768], F16)
        ident_t = cload(ident_h, [128, 128], F16)
        wq_t = cload(wq_h, [128, 2, 256], F16)
        ssel_t = cload(ssel_h, [128, 32], F16)
        s2sel_t = cload(s2sel_h, [32, 128], F16)
        wgc_t = cload(wgc_h, [128, 32], F32)
        wge_t = cload(wge_h, [128, 32], F32)
        wgav_t = cload(wgav_h, [128, 5], F16)
        if with_bias:
            bcat_t = cload(bcat_h, [1, 3, 256], F16)
            bq_t = cload(bq_h, [1, 256], F16)
            ones_t = cload(ones_h, [1, 128], F16)

        qdram = dram.tile([BM, A * D], F32)
        pend = {}

        # ---- all q projections up front (natural layout) -> DRAM ----
        for mol in range(BM):
            qin16 = sb_q.tile([128, 256], F16, tag="qin16")
            nc.gpsimd.dma_start(out=qin16, in_=qin_h[mol])
            qtp = ps_tp.tile([128, 2, 128], F16, tag="pt", name="qtp")
            for w in range(2):
                nc.tensor.transpose(qtp[:, w, :],
                                    qin16[:, 128 * w:128 * (w + 1)], ident_t)
            qT = sb_q.tile([128, 2, 128], F16, tag="qT")
            nc.scalar.copy(out=qT, in_=qtp)
            qpsum = ps_misc.tile([128, 256], F32, tag="pm")
            nc.tensor.matmul(qpsum, qT[:, 0, :], wq_t[:, 0, :],
                             start=True, stop=False)
            nc.tensor.matmul(qpsum, qT[:, 1, :], wq_t[:, 1, :],
                             start=False, stop=not with_bias)
            if with_bias:
                nc.tensor.matmul(qpsum, ones_t, bq_t, start=False, stop=True)
            qnat = sb_q.tile([128, 256], F32, tag="qnat")
            nc.scalar.copy(out=qnat, in_=qpsum)
            nc.scalar.dma_start(
                out=qdram[mol].rearrange("(a c) -> a c", a=128), in_=qnat)

        for mol in range(BM):
            q_read = qdram[mol].rearrange(
                "(g p r k) -> p g r k", g=2, p=128, r=4, k=32)
            cur4m = sb_ew.tile([128, 2, 4, 32], F32, tag="cur4")
            nc.scalar.dma_start(
                out=cur4m,
                in_=q5[mol].rearrange("g p r k -> p g r k"))
            qu4m = sb_ew.tile([128, 2, 4, 32], F32, tag="qu4")
            nc.scalar.dma_start(out=qu4m, in_=q_read)

            for G in range(2):
                # ---- X load (cast f32->f16 in DMA) + PE transpose, with
                # 5 chunks batched per PSUM bank and one grouped copy ----
                xbig = sb_x16.tile([128, 2560], F16, tag="x16")
                nc.gpsimd.dma_start(out=xbig, in_=x5[mol, G])
                xu16 = [xbig[:, 640 * r:640 * (r + 1)] for r in range(4)]
                XT = {}
                for r in range(4):
                    tp = ps_tp.tile([128, 5, 128], F16, tag="pt", name="tp")
                    for w in range(5):
                        nc.tensor.transpose(
                            tp[:, w, :], xu16[r][:, 128 * w:128 * (w + 1)],
                            ident_t)
                    xtb = sb_xt.tile([128, 5, 128], F16, tag="xt")
                    nc.scalar.copy(out=xtb, in_=tp)
                    for w in range(5):
                        d, fc = divmod(5 * r + w, 4)
                        XT[(d, fc)] = xtb[:, w, :]

                cur4 = cur4m[:, G]
                qu4 = qu4m[:, G]

                arawB = sb_stash.tile([128, 4, 32], F32, tag="arawB")
                emaxB = sb_ew.tile([128, 4, 32], F32, tag="emaxB")
                gaveB = sb_ew.tile([128, 4], F32, tag="gaveB")
                adenB = sb_ew.tile([128, 4], F32, tag="adenB")
                ex4 = sb_ew.tile([128, 4, 10], F16, tag="ex4", name="ex4")
                pg = ps_misc.tile([128, 4], F32, tag="pm", name="pg")

                for r in range(4):
                    # ---- projections K|V|M into one 3-bank psum tile ----
                    wA = 256 - 64 * r
                    ranges = [(r, 0, wA, 64 * r), (r + 1, wA, 320 - wA, 0)]
                    kvm = ps_proj.tile([128, 3, 320], F32, tag="pp",
                                       padded_shape=[128, 3, 512])
                    for (d, t0, wd, e0) in ranges:
                        for fc in range(4):
                            st = fc == 0
                            sp = (fc == 3) and not with_bias
                            for i in range(3):
                                nc.tensor.matmul(
                                    kvm[:, i, t0:t0 + wd], XT[(d, fc)],
                                    wcat_t[:, fc, 256 * i + e0:256 * i + e0 + wd],
                                    start=st, stop=sp)
                            # fold the neighbor-mean dot into the PE pass
                            w_ave = 4 * d + fc - 5 * r
                            if 0 <= w_ave < 5:
                                nc.tensor.matmul(
                                    pg[:, r:r + 1], XT[(d, fc)],
                                    wgav_t[:, w_ave:w_ave + 1],
                                    start=(w_ave == 0), stop=(w_ave == 4),
                                    skip_group_check=True)
                        if with_bias:
                            for i in range(3):
                                nc.tensor.matmul(
                                    kvm[:, i, t0:t0 + wd], ones_t,
                                    bcat_t[:, i, e0:e0 + wd],
                                    start=False, stop=True)

                    # ---- per-phase elementwise ----
                    smul = sb_big.tile([128, 320], F16, tag="smul")
                    nc.vector.tensor_mul(
                        smul, kvm[:, 0, 0:320],
                        qu4[:, r, :].unsqueeze(1).broadcast_to([128, 10, 32]))
                    score = sb_ew.tile([128, 10], F32, tag="score")
                    nc.vector.tensor_reduce(
                        out=score, in_=smul.rearrange("p (j k) -> p j k", j=10),
                        axis=AXL_X, op=ADD)
                    ex = ex4[:, r, :]
                    nc.scalar.activation(out=ex, in_=score, func=EXP)
                    v16 = sb_big.tile([128, 320], F16, tag="v16")
                    nc.scalar.copy(out=v16, in_=kvm[:, 1, 0:320])
                    amul = sb_big.tile([128, 320], F16, tag="amul")
                    nc.gpsimd.tensor_mul(
                        amul, v16,
                        ex.unsqueeze(2).broadcast_to([128, 10, 32]))
                    nc.vector.tensor_reduce(
                        out=arawB[:, r, :],
                        in_=amul.rearrange("p (j k) -> p k j", j=10),
                        axis=AXL_X, op=ADD)
                    nc.vector.tensor_reduce(
                        out=emaxB[:, r, :],
                        in_=kvm[:, 2, 0:320].rearrange("p (j k) -> p k j", j=10),
                        axis=AXL_X, op=MAX)
                    if DEBUG and mol == 0 and G == 0 and r == 0:
                        kc = sb_big.tile([128, 320], F32, tag="dbgk")
                        nc.vector.tensor_copy(out=kc, in_=kvm[:, 0, :])
                        nc.sync.dma_start(out=dbg["dbg_k"][:, :320], in_=kc)
                        vc = sb_big.tile([128, 320], F32, tag="dbgv")
                        nc.vector.tensor_copy(out=vc, in_=kvm[:, 1, :320])
                        nc.sync.dma_start(out=dbg["dbg_v"][:], in_=vc)
                        mc = sb_big.tile([128, 320], F32, tag="dbgm")
                        nc.vector.tensor_copy(out=mc, in_=kvm[:, 2, :320])
                        nc.sync.dma_start(out=dbg["dbg_m"][:], in_=mc)
                        nc.sync.dma_start(out=dbg["dbg_score"][:], in_=score)
                        xtc = sb_big.tile([128, 128], F32, tag="dbgxt")
                        nc.vector.tensor_copy(out=xtc, in_=XT[(0, 0)])
                        nc.sync.dma_start(out=dbg["dbg_xt"][:], in_=xtc)

                # ---- gate logits (batched over the 4 phases) ----
                nc.vector.tensor_reduce(out=adenB, in_=ex4, axis=AXL_X, op=ADD)
                nc.vector.tensor_copy(out=gaveB, in_=pg)
                curp = sb_ew.tile([128, 4, 32], F32, tag="curp")
                nc.gpsimd.tensor_mul(
                    curp, cur4,
                    wgc_t.unsqueeze(1).broadcast_to([128, 4, 32]))
                gcurB = sb_ew.tile([128, 4], F32, tag="gcurB")
                nc.vector.tensor_reduce(out=gcurB, in_=curp, axis=AXL_X, op=ADD)
                emaxp = sb_ew.tile([128, 4, 32], F32, tag="emaxp")
                nc.gpsimd.tensor_mul(
                    emaxp, emaxB,
                    wge_t.unsqueeze(1).broadcast_to([128, 4, 32]))
                gemxB = sb_ew.tile([128, 4], F32, tag="gemxB")
                nc.vector.tensor_reduce(out=gemxB, in_=emaxp, axis=AXL_X, op=ADD)
                gl1 = sb_ew.tile([128, 4], F32, tag="gl1")
                nc.vector.tensor_add(gl1, gcurB, gemxB)
                gl2 = sb_ew.tile([128, 4], F32, tag="gl2")
                nc.vector.tensor_add(gl2, gl1, gaveB)
                egB = sb_stash.tile([128, 4], F32, tag="egB")
                nc.scalar.activation(out=egB, in_=gl2, func=EXP,
                                     bias=float(bg_val))
                egB16 = sb_stash.tile([128, 4], F16, tag="egB16")
                nc.vector.tensor_copy(out=egB16, in_=egB)
                raB = sb_stash.tile([128, 4], F32, tag="raB")
                nc.vector.reciprocal(out=raB, in_=adenB)

                if DEBUG and mol == 0 and G == 0:
                    nc.sync.dma_start(out=dbg["dbg_araw"][:], in_=arawB)
                    nc.sync.dma_start(out=dbg["dbg_emax"][:], in_=emaxB)
                    nc.sync.dma_start(out=dbg["dbg_gave"][:], in_=gaveB)
                    egc = sb_ew.tile([128, 4], F32, tag="dbgeg")
                    nc.vector.tensor_copy(out=egc, in_=egB)
                    nc.sync.dma_start(out=dbg["dbg_eg"][:], in_=egc)
                    nc.sync.dma_start(out=dbg["dbg_aden"][:], in_=adenB)
                    nc.sync.dma_start(out=dbg["dbg_qu"][:], in_=qu4)

                if G == 0:
                    st0 = (arawB, egB, egB16, raB)
                else:
                    pend[mol] = (st0, (arawB, egB, egB16, raB))

            # gate softmax + output for the previous molecule: deferred one
            # mol so the PE never waits on the current DVE softmax chain
            for fm in ([mol - 1] if mol >= 1 else []) + \
                    ([BM - 1] if mol == BM - 1 else []):
                if fm not in pend:
                    continue
                fst0, fst1 = pend.pop(fm)
                gd = ps_misc.tile([32, 4], F32, tag="pm", name="gd")
                for r in range(4):
                    nc.tensor.matmul(gd[:, r:r + 1], ssel_t,
                                     fst0[2][:, r:r + 1],
                                     start=True, stop=False)
                    nc.tensor.matmul(gd[:, r:r + 1], ssel_t,
                                     fst1[2][:, r:r + 1],
                                     start=False, stop=True)
                rg = sb_ew.tile([32, 4], F32, tag="rg")
                nc.vector.reciprocal(out=rg, in_=gd)
                rg16 = sb_ew.tile([32, 4], F16, tag="rg16")
                nc.vector.tensor_copy(out=rg16, in_=rg)
                inv = ps_misc.tile([128, 4], F32, tag="pm", name="inv")
                for r in range(4):
                    nc.tensor.matmul(inv[:, r:r + 1], s2sel_t,
                                     rg16[:, r:r + 1],
                                     start=True, stop=True)
                c2B = {}
                for gg, (ar_g, eg_g, eg16_g, ra_g) in (
                        (0, fst0), (1, fst1)):
                    t1 = sb_ew.tile([128, 4], F32, tag="t1", name="t1")
                    nc.vector.tensor_mul(t1, inv, ra_g)
                    c2B[gg] = sb_stash.tile([128, 4], F32, tag=f"c2B{gg}",
                                            name=f"c2B{gg}")
                    nc.vector.tensor_mul(c2B[gg], t1, eg_g)
                for gg, ar_g in ((0, fst0[0]), (1, fst1[0])):
                    outB = sb_ew.tile([128, 4, 32], F32, tag="outB")
                    nc.gpsimd.tensor_mul(
                        outB, ar_g,
                        c2B[gg].unsqueeze(2).broadcast_to([128, 4, 32]))
                    nc.scalar.dma_start(out=o5[fm, gg], in_=outB)
    nc.finalize()
    return nc


def _prep_consts(Wq, bq, Wk, bk, Wv, bv, Wam, bam, Wg, bg):
    wcat = np.empty((128, 4, 768), np.float16)
    for i, W in enumerate((Wk, Wv, Wam)):
        for fc in range(4):
            wcat[:, fc, 256 * i:256 * (i + 1)] = W[128 * fc:128 * (fc + 1), :]
    wq = np.empty((128, 2, 256), np.float16)
    for fc in range(2):
        wq[:, fc, :] = Wq[128 * fc:128 * (fc + 1), :]
    p = np.arange(128)
    ssel = (p[:, None] % 32 == np.arange(32)[None, :]).astype(np.float16)
    s2sel = ssel.T.copy()
    wg = np.asarray(Wg[:, 0], np.float32)
    # wg_avc[floc, w] = Wg[64 + ((128*w + floc) % 64)] / NEI
    wgav = np.empty((128, 5), np.float32)
    for w in range(5):
        wgav[:, w] = wg[64 + (np.arange(128) % 64)] / NEI
    consts = {
        "wcat": wcat, "wq": wq,
        "ident": np.eye(128, dtype=np.float16),
        "ssel": ssel, "s2sel": s2sel,
        "wg_cur": np.tile(wg[0:32], (128, 1)).astype(np.float32),
        "wg_emax": np.tile(wg[32:64], (128, 1)).astype(np.float32),
        "wg_avc": wgav.astype(np.float16),
    }
    with_bias = any(np.any(np.asarray(b) != 0) for b in (bq, bk, bv, bam))
    if with_bias:
        bcat = np.stack([np.asarray(bk), np.asarray(bv), np.asarray(bam)]
                        ).astype(np.float16)[None, :, :].reshape(1, 3, 256)
        consts["bcat"] = bcat
        consts["bq"] = np.asarray(bq, np.float16).reshape(1, 256)
        consts["ones"] = np.ones((1, 128), np.float16)
    return consts, with_bias, float(np.asarray(bg).reshape(-1)[0])


_CACHE = {}
TRACE = False       # set by test.py for profiling runs
LAST_RESULTS = None  # BassKernelResults from the most recent run


def kernel(input_multihead, input_q, Wq, bq, Wk, bk, Wv, bv, Wam, bam, Wg, bg):
    from concourse.bass_utils import run_bass_kernel_spmd

    consts, with_bias, bg_val = _prep_consts(
        Wq, bq, Wk, bk, Wv, bv, Wam, bam, Wg, bg)

    key = (with_bias, bg_val)
    if key not in _CACHE:
        _CACHE[key] = build_nc(with_bias, bg_val)
    nc = _CACHE[key]

    x = np.ascontiguousarray(np.asarray(input_multihead, np.float32))
    q = np.ascontiguousarray(np.asarray(input_q, np.float32))
    in_maps = []
    for c in range(N_CORES):
        m = {"x": x[BM * c:BM * (c + 1)], "qin": q[BM * c:BM * (c + 1)]}
        m.update(consts)
        in_maps.append(m)

    res = run_bass_kernel_spmd(nc, in_maps, list(range(N_CORES)), trace=TRACE)
    global LAST_RESULTS
    LAST_RESULTS = res
    return np.concatenate([res.results[c]["out"] for c in range(N_CORES)],
                          axis=0)

